# revision 11
# baseline (speedup 1.0000x reference)
"""Distributed multi-head attention for TRN2, 8 NeuronCores.

Sharding: tensor-parallel over heads (2 heads / core) for QKV + attention;
an AllToAll exchanges normalized attention outputs so each core computes
the output projection for its own 512 sequence rows.

v2 vs baseline (454us):
- Head: few big input DMAs spread over sync/scalar/gpsimd rings (issue cost
  was ~60us of serialized 650ns issues); K-proj streams d-outer behind the
  x DMA; attention starts right after kT + first qT chunk (~30us).
- V projected in transposed layout (weights stationary, f=512) then moved
  to natural layout by xbar DMA transposes -> 14us of PE instead of 63us.
- Scores matmuls contract only dh=64, so the two heads' matmuls are issued
  back-to-back into disjoint PE row groups (tile_position auto from base
  partition) and run concurrently: GS=2 groups both heads of one k-tile.
- Softmax exp was the bottleneck (ACT 1 elem/cycle/lane = 264us busy).
  ~40% of groups compute exp on the DVE instead, as an exp2 bit trick:
  u8 = round(a*s + b) IS the fp8-e4m3 bit pattern of ~exp(s*scale)
  (round-half-even + saturate-at-0 verified on HW). The PV matmul takes
  the fp8 rhs against bf16 V directly (mixed-dtype matmul verified).
  The ones-row in V computes denominators from the same approximated
  weights, so softmax normalization stays exact.
"""
import numpy as np
import ml_dtypes

import concourse.bass as bass
import concourse.tile as tile
from concourse import bacc, mybir
from concourse.bass_utils import run_bass_kernel_spmd
from concourse.masks import make_identity

# problem dims (hardcoded; kernel.py must be self-contained)
N, DIM, HEADS, DH = 4096, 1024, 16, 64
NCORES = 8
HPC = HEADS // NCORES        # 2 heads per core
ICB = HPC * DH               # 128 inner dims per core
DCH = DIM // 128             # 8 dim chunks
QC = 512                     # query-chunk (columns per scores matmul)
NQ = N // QC                 # 8
KT = 128                     # key tile (scores output partitions)
NKT = N // KT                # 32
SEQC = N // NCORES           # 512 output rows per core
SCALE = float(DH) ** -0.5

BF16 = mybir.dt.bfloat16
F32 = mybir.dt.float32
F8 = mybir.dt.float8e4
U8 = mybir.dt.uint8
BF16_NP = ml_dtypes.bfloat16
F8_NP = ml_dtypes.float8_e4m3

# how many of each 32 k-tiles take the DVE fast-exp path (rest use ACT)
DVE_FRAC = 14
# PV for group g is emitted PEND_D groups later: the PE runs in order, so
# this keeps the exp latency of group g off the PE critical path.
PEND_D = 4


def _dve_t(t):
    return t < 2 * DVE_FRAC and t % 2 == 0


# exp2 bit-trick constants: u16 = round(EXP_A * s + EXP_B) is the bf16 bit
# pattern of approximately exp(SCALE * s) (Schraudolph).  EXP_B calibrated so
# the multiplicative error is mean-zero over the logit distribution
# (sigma~2.7), keeping DVE-slot weights consistent with ACT-slot exact exps.
EXP_A = 128.0 * np.log2(np.e) * SCALE


def _calibrate_exp_b():
    rng = np.random.default_rng(7)
    s = rng.normal(0.0, 2.67, 100000)
    exact = np.exp(SCALE * s)
    best_b, best_bias = None, None
    for c in np.linspace(-0.10, 0.02, 121):
        b = 128.0 * (127.0 + c)
        u = np.clip(np.round(EXP_A * s + b), 0, 65535).astype(np.uint16)
        w = u.view(BF16_NP).astype(np.float64)
        bias = abs(np.mean(w / exact) - 1.0)
        if best_bias is None or bias < best_bias:
            best_b, best_bias = b, bias
    return float(best_b)


EXP_B = _calibrate_exp_b()


def build_kernel():
    nc = bacc.Bacc("TRN2", target_bir_lowering=False, debug=False,
                   enable_asserts=True, num_devices=NCORES)

    xt = nc.dram_tensor("xt", [128, DCH, N], BF16, kind="ExternalInput")
    wq = nc.dram_tensor("wq", [128, DCH, ICB], BF16, kind="ExternalInput")
    wk = nc.dram_tensor("wk", [128, DCH, ICB], BF16, kind="ExternalInput")
    wv = nc.dram_tensor("wv", [128, DCH, ICB], BF16, kind="ExternalInput")
    wo = nc.dram_tensor("wo", [128, DCH, DIM], BF16, kind="ExternalInput")
    bo = nc.dram_tensor("bo", [128, DIM], F32, kind="ExternalInput")
    out = nc.dram_tensor("out", [SEQC, DIM], F32, kind="ExternalOutput")
    wsink = nc.dram_tensor("warm_sink", [128, 16], F32, kind="ExternalOutput")

    with tile.TileContext(nc) as tc:
        with (
            tc.tile_pool(name="xtp", bufs=1) as xtp,
            tc.tile_pool(name="wp", bufs=1) as wp,
            tc.tile_pool(name="qk", bufs=1) as qkp,
            tc.tile_pool(name="dram", bufs=1, space="DRAM") as dramp,
        ):
            # bar_i first on an empty gpsimd DMA queue (a tiny transfer queued
            # behind megabytes would stall the early barrier for ~40us)
            bar_i = dramp.tile([1, 16], F32, tag="bar_i")
            bar_o = dramp.tile([1, 16], F32, tag="bar_o", addr_space="Shared")
            nc.gpsimd.dma_start(bar_i[:], bo[0:1, 0:16])

            # ---- input DMAs: few big transfers, spread across issue rings.
            # wk first (first consumer), then x; wq/wv/wo/bo later.
            wq_t = wp.tile([128, DCH, ICB], BF16, tag="wq")
            wk_t = wp.tile([128, DCH, ICB], BF16, tag="wk")
            wv_t = wp.tile([128, DCH, ICB], BF16, tag="wv")
            wo_t = wp.tile([128, DCH, DIM], BF16, tag="wo")
            bo_t = wp.tile([128, DIM], F32, tag="bo")
            nc.sync.dma_start(wk_t[:], wk[:])
            xt_t = [xtp.tile([128, N], BF16, tag=f"xt{d}", name=f"xt{d}")
                    for d in range(DCH)]
            rings = [nc.sync, nc.scalar, nc.gpsimd]
            k = 0
            for d in range(DCH):
                for half in range(2):
                    sl = slice(half * (N // 2), (half + 1) * (N // 2))
                    rings[k % 2].dma_start(xt_t[d][:, sl], xt[:, d, sl])
                    k += 1
            nc.scalar.dma_start(wq_t[:], wq[:])
            nc.scalar.dma_start(wv_t[:], wv[:])
            nc.scalar.dma_start(wo_t[:], wo[:])
            nc.scalar.dma_start(bo_t[:], bo[:])

            qT = qkp.tile([128, N], BF16, tag="qT")   # [2 heads x 64, seq]
            kT = qkp.tile([128, N], BF16, tag="kT")
            vT = qkp.tile([128, N], BF16, tag="vT")   # transposed V
            # natural-layout V + ones column: [seq-tile part, kt, h, DH+1]
            vt = qkp.tile([128, NKT, HPC, DH + 1], BF16, tag="vt")
            wz = wp.tile([128, QC], BF16, tag="wz")
            nc.gpsimd.memset(wz[:], 0.0)   # first: gates the PE warm-up
            nc.gpsimd.memset(vt[:], 1.0)
            ident = qkp.tile([128, 128], BF16, tag="ident")
            make_identity(nc, ident[:])

            # early barrier: absorb inter-core startup skew while hidden.
            # Emitted after the memset so the gpsimd ring work that gates the
            # PE warm-up isn't blocked behind the barrier wait.
            nc.gpsimd.collective_compute(
                "AllReduce", mybir.AluOpType.add,
                replica_groups=[list(range(NCORES))],
                ins=[bar_i.opt()], outs=[bar_o.opt()],
            )

            a2a_in = dramp.tile([NCORES, ICB, QC], BF16, tag="a2a_in")
            a2a_out = dramp.tile([NCORES, ICB, QC], BF16, tag="a2a_out")

            # ---- K projection, d-outer across all 8 chunks so matmuls
            # trail the x DMA stream (8 PSUM banks) ----
            with tc.tile_pool(name="psA", bufs=8, space="PSUM") as psA:
                # HAM warm-up: dep-free matmuls while DMAs stream
                w_ps = psA.tile([128, QC], F32, tag="proj", name="warm_ps")
                last_warm = None
                for _ in range(48):
                    last_warm = nc.tensor.matmul(w_ps[:], wz[:, 0:128], wz[:],
                                                 start=True, stop=True)
                wcp = wp.tile([128, 16], F32, tag="wcp")
                nc.vector.tensor_copy(wcp[:], w_ps[:, 0:16])
                nc.sync.dma_start(wsink[:], wcp[:])

                # chunks 0-6 ride the x DMA stream (7 banks); warm filler
                # matmuls between d-groups keep HAM at full clock through the
                # DMA-wait gaps; chunk 7 runs dense at stream end.
                first_real = None
                ps = [psA.tile([128, QC], F32, tag="proj", name=f"kps{j}")
                      for j in range(NQ - 1)]
                for d in range(DCH):
                    for j in range(NQ - 1):
                        m = nc.tensor.matmul(
                            ps[j][:], wk_t[:, d, :], xt_t[d][:, j * QC:(j + 1) * QC],
                            start=(d == 0), stop=(d == DCH - 1))
                        if first_real is None:
                            first_real = m
                    for _ in range(4):
                        nc.tensor.matmul(w_ps[:], wz[:, 0:128], wz[:],
                                         start=True, stop=True)
                ps7 = psA.tile([128, QC], F32, tag="proj", name="kps7")
                for d in range(DCH):
                    nc.tensor.matmul(
                        ps7[:], wk_t[:, d, :],
                        xt_t[d][:, (NQ - 1) * QC:NQ * QC],
                        start=(d == 0), stop=(d == DCH - 1))
                for j in range(NQ - 1):
                    nc.vector.tensor_copy(kT[:, j * QC:(j + 1) * QC], ps[j][:])
                nc.vector.tensor_copy(kT[:, (NQ - 1) * QC:NQ * QC], ps7[:])
                bass._add_dep_helper(first_real.ins, last_warm.ins, sync=False,
                                     reason="warm-up runs before projections")

            # ---- Q (pair-wise) and V-transposed (chunk-wise) projections.
            # Emission order = PE priority: q chunk 0 first so attention can
            # start, then vT chunks early (PV needs vt tiles soon after).
            with tc.tile_pool(name="psB", bufs=1, space="PSUM") as psB:
                def q_pair(p):
                    pq = [psB.tile([128, QC], F32, tag="qproj", bufs=3,
                                   name=f"qps{p}_{i}") for i in range(2)]
                    for d in range(DCH):
                        for i in range(2):
                            nc.tensor.matmul(
                                pq[i][:], wq_t[:, d, :],
                                xt_t[d][:, (2 * p + i) * QC:(2 * p + i + 1) * QC],
                                start=(d == 0), stop=(d == DCH - 1))
                    for i in range(2):
                        j = 2 * p + i
                        nc.vector.tensor_copy(qT[:, j * QC:(j + 1) * QC], pq[i][:])

                def v_chunk(c):
                    pv_ = psB.tile([128, QC], F32, tag="vproj", bufs=3,
                                   name=f"vps{c}")
                    for d in range(DCH):
                        nc.tensor.matmul(
                            pv_[:], wv_t[:, d, :], xt_t[d][:, c * QC:(c + 1) * QC],
                            start=(d == 0), stop=(d == DCH - 1))
                    nc.vector.tensor_copy(vT[:, c * QC:(c + 1) * QC], pv_[:])
                    # natural layout via PE transpose-mode + one strided copy
                    for tt in range(4):
                        t = 4 * c + tt
                        pst = psB.tile([128, 128], BF16, tag="vtr", bufs=2,
                                       name=f"vtr{t}")
                        nc.tensor.transpose(pst[:], vT[:, t * KT:(t + 1) * KT],
                                            ident[:])
                        nc.vector.tensor_copy(
                            vt[:, t, :, 0:DH],
                            pst[:].rearrange("p (a b) -> p a b", a=2))

                q_pair(0)
                v_chunk(0)
                v_chunk(1)
                q_pair(1)
                v_chunk(2)
                v_chunk(3)
                q_pair(2)
                v_chunk(4)
                v_chunk(5)
                q_pair(3)
                v_chunk(6)
                v_chunk(7)

            with (
                tc.tile_pool(name="psS", bufs=3, space="PSUM") as psS,
                tc.tile_pool(name="psV", bufs=2, space="PSUM") as psV,
                tc.tile_pool(name="expp", bufs=6) as expp,
                tc.tile_pool(name="ex8p", bufs=6) as ex8p,
                tc.tile_pool(name="attp", bufs=4) as attp,
                tc.tile_pool(name="invp", bufs=3) as invp,
            ):
                # ---- attention: groups of (k-tile t) x (both heads); the two
                # scores matmuls go to disjoint PE row groups and overlap.
                pv = {}
                pend = []

                def emit_pv(j, t, ex, is8):
                    for h in range(HPC):
                        rhs = ex[:, h, :].bitcast(BF16) if is8 else ex[:, h, :]
                        nc.tensor.matmul(
                            pv[j][h][0:DH + 1, :],
                            vt[:, t, h, :],
                            rhs,
                            start=(t == 0), stop=(t == NKT - 1),
                        )

                def emit_epilogue(j):
                    den = [invp.tile([1, QC], F32, tag="den", name=f"den{j}_{h}")
                           for h in range(HPC)]
                    inv = [invp.tile([1, QC], F32, tag="inv", name=f"inv{j}_{h}")
                           for h in range(HPC)]
                    invb = [invp.tile([DH, QC], F32, tag="invb", name=f"invb{j}_{h}")
                            for h in range(HPC)]
                    an = [attp.tile([DH, QC], BF16, tag="an", name=f"an{j}_{h}")
                          for h in range(HPC)]
                    # recip_approx_fast misreads PSUM sources; stage via SBUF
                    nc.vector.tensor_copy(den[0][:], pv[j][0][DH:DH + 1, :])
                    nc.vector.reciprocal_approx_fast(inv[0][:], den[0][:])
                    nc.gpsimd.partition_broadcast(invb[0][:], inv[0][:])
                    nc.vector.tensor_copy(den[1][:], pv[j][1][DH:DH + 1, :])
                    nc.vector.reciprocal_approx_fast(inv[1][:], den[1][:])
                    nc.vector.tensor_mul(an[0][:], pv[j][0][0:DH, :], invb[0][:])
                    nc.gpsimd.partition_broadcast(invb[1][:], inv[1][:])
                    nc.vector.tensor_mul(an[1][:], pv[j][1][0:DH, :], invb[1][:])
                    for h in range(HPC):
                        nc.sync.dma_start(a2a_in[j, h * DH:(h + 1) * DH, :], an[h][:])
                    del pv[j]
                    if j == NQ - 2:
                        # re-sync cores while the last q-chunk computes
                        bar2_i = dramp.tile([1, 16], F32, tag="bar2_i")
                        bar2_o = dramp.tile([1, 16], F32, tag="bar2_o",
                                            addr_space="Shared")
                        nc.gpsimd.dma_start(bar2_i[:], a2a_in[j, 0:1, 0:16])
                        nc.gpsimd.collective_compute(
                            "AllReduce", mybir.AluOpType.add,
                            replica_groups=[list(range(NCORES))],
                            ins=[bar2_i.opt()], outs=[bar2_o.opt()],
                        )

                def flush_one():
                    jj, tt, exx, is8 = pend.pop(0)
                    emit_pv(jj, tt, exx, is8)
                    if tt == NKT - 1:
                        emit_epilogue(jj)

                for j in range(NQ):
                    pv[j] = [psV.tile([128, QC], F32, tag="pv", name=f"pv{j}_{h}")
                             for h in range(HPC)]
                    for t in range(NKT):
                        sc = psS.tile([128, HPC, QC], F32, tag="sc")
                        for h in range(HPC):
                            nc.tensor.matmul(
                                sc[:, h, :],
                                kT[h * DH:(h + 1) * DH, t * KT:(t + 1) * KT],
                                qT[h * DH:(h + 1) * DH, j * QC:(j + 1) * QC],
                                start=True, stop=True,
                            )
                        if _dve_t(t):
                            ex = ex8p.tile([128, HPC, QC], mybir.dt.uint16,
                                           tag="ex8")
                            nc.vector.tensor_scalar(
                                ex[:], sc[:], EXP_A, EXP_B,
                                mybir.AluOpType.mult, mybir.AluOpType.add)
                            pend.append((j, t, ex, True))
                        else:
                            ex = expp.tile([128, HPC, QC], BF16, tag="ex")
                            nc.scalar.activation(ex[:], sc[:],
                                                 mybir.ActivationFunctionType.Exp,
                                                 scale=SCALE)
                            pend.append((j, t, ex, False))
                        if len(pend) > PEND_D:
                            flush_one()
                while pend:
                    flush_one()

            # ---- exchange: my (2 heads x all seq) -> (all inner x my seq) ----
            nc.gpsimd.collective_compute(
                "AllToAll", mybir.AluOpType.bypass,
                replica_groups=[list(range(NCORES))],
                ins=[a2a_in.opt()], outs=[a2a_out.opt()],
            )

            # ---- output projection for my SEQC rows ----
            with (
                tc.tile_pool(name="psC", bufs=2, space="PSUM") as psC,
                tc.tile_pool(name="finp", bufs=3) as finp,
            ):
                # dep-free matmuls keep HAM at full clock through the
                # AllToAll wait so the output projection runs warm
                w2_ps = psC.tile([128, QC], F32, tag="warm2", bufs=1)
                for _ in range(90):
                    nc.tensor.matmul(w2_ps[:], wz[:, 0:128], wz[:],
                                     start=True, stop=True)
                af = finp.tile([128, NCORES, QC], BF16, tag="af")
                for r in range(NCORES):
                    rings[r % 3].dma_start(af[:, r, :], a2a_out[r])
                bo3 = bo_t[:].rearrange("p (a b) -> p a b", a=2)
                for s in range(SEQC // 128):
                    yps = psC.tile([128, 2, QC], F32, tag="y")
                    for r in range(NCORES):
                        for half in range(2):
                            nc.tensor.matmul(
                                yps[:, half, :],
                                af[:, r, s * 128:(s + 1) * 128],
                                wo_t[:, r, half * QC:(half + 1) * QC],
                                start=(r == 0), stop=(r == NCORES - 1))
                    ysb = finp.tile([128, 2, QC], F32, tag="ysb")
                    nc.vector.tensor_add(ysb[:], yps[:], bo3)
                    orows = out[s * 128:(s + 1) * 128, :].rearrange(
                        "p (a b) -> p a b", a=2)
                    for half in range(2):
                        for pp in range(2):
                            ring = nc.sync if (half + pp) % 2 == 0 else nc.scalar
                            ring.dma_start(
                                orows[:, half, pp * 256:(pp + 1) * 256],
                                ysb[:, half, pp * 256:(pp + 1) * 256])

    nc.compile()
    return nc


_NC_CACHE = None


def _get_nc():
    global _NC_CACHE
    if _NC_CACHE is None:
        _NC_CACHE = build_kernel()
    return _NC_CACHE


def _prep_inputs(x, Wq, Wk, Wv, Wo, bo):
    """Host-side sharding/layout prep (untimed)."""
    xt_p = np.ascontiguousarray(
        x.T.reshape(DCH, 128, N).transpose(1, 0, 2)).astype(BF16_NP)
    wo_p = np.ascontiguousarray(
        Wo.reshape(DCH, 128, DIM).transpose(1, 0, 2)).astype(BF16_NP)
    bo_p = np.ascontiguousarray(np.tile(bo[None, :], (128, 1))).astype(np.float32)
    in_maps = []
    for c in range(NCORES):
        ic = slice(c * ICB, (c + 1) * ICB)
        m = {"xt": xt_p, "wo": wo_p, "bo": bo_p}
        for name, W in (("wq", Wq), ("wk", Wk), ("wv", Wv)):
            m[name] = np.ascontiguousarray(
                W[:, ic].reshape(DCH, 128, ICB).transpose(1, 0, 2)).astype(BF16_NP)
        in_maps.append(m)
    return in_maps


def kernel(x, Wq, Wk, Wv, Wo, bo, _trace=False):
    x = np.asarray(x, np.float32)
    Wq = np.asarray(Wq, np.float32)
    Wk = np.asarray(Wk, np.float32)
    Wv = np.asarray(Wv, np.float32)
    Wo = np.asarray(Wo, np.float32)
    bo = np.asarray(bo, np.float32)
    nc = _get_nc()
    in_maps = _prep_inputs(x, Wq, Wk, Wv, Wo, bo)
    r = run_bass_kernel_spmd(nc, in_maps, core_ids=list(range(NCORES)),
                             trace=_trace)
    y = np.concatenate([r.results[c]["out"] for c in range(NCORES)], axis=0)
    if _trace:
        kernel.last_result = r
    return y.astype(np.float32)
